# revision 5
# baseline (speedup 1.0000x reference)
"""Trainium2 Bass kernel for the leaky-tanh recurrent network (CombinedModel).

Math (reference):
    u_seq = u.transpose(0,2,1).reshape(Iq*T, C)      # state persists across inquiries
    x_t = (1-dt)*x_{t-1} + (dt*W) @ tanh(x_{t-1}) + (dt*M) @ u_t
    y_t = sigmoid(H @ x_t + b)
    outputs, membranes = y, x   (each reshaped back to [Iq, C, T])

Strategy: the recurrence is sequential in time, but it can be solved in
blocks of B steps with Picard (fixed-point) iteration:
    - the input drive V = (dt*M) @ U is a big batched matmul (TensorE)
    - given a guess trajectory X, G = (dt*W) @ tanh(X_shifted) is batched
    - the affine recurrence x_t = a*x_{t-1} + (G_t + V_t) is ONE hardware
      instruction per 128 channels via the DVE's tensor_tensor_scan
    - iterate K times; measured numerically: abs err 2.4e-5 at K=8, B=128
      vs the exact sequential scan
Everything stays in [channel, time] layout on chip; DMA access patterns
handle the [Iq, C, T] <-> [C, time] mapping directly, so no transposes.

The per-iteration serial chain (scan -> tanh -> matmul -> scan) is
pipelined at half-channel granularity: tanh/matmuls for channels 0-127
run while the scan/tanh for channels 128-255 are still in flight, and the
drive-injection matmul (I @ V, which has no data dependence on the
iteration) opens each PSUM accumulation group early.  The V-phase of the
next block and the readout of the previous block are emitted in the
middle of the current block's iteration stretch so the in-order engines
execute them inside the chain's dependency stalls.
"""

import numpy as np

INQ, C, T = 64, 256, 1024
B = 128            # picard block length (divides T)
KIT = 8            # picard refinement iterations per block
P = 128            # partitions; C = 2*P channel halves

_cache: dict = {}


def _build(dt: float, n_blocks: int, kit: int):
    import concourse.bacc as bacc
    import concourse.mybir as mybir
    from concourse.tile import TileContext

    f32 = mybir.dt.float32
    AF = mybir.ActivationFunctionType
    ALU = mybir.AluOpType
    a = 1.0 - dt

    nc = bacc.Bacc(None, target_bir_lowering=False, debug=False)
    u_in = nc.dram_tensor("u", [INQ, C, T], f32, kind="ExternalInput")
    wt_in = nc.dram_tensor("wt", [2, P, C], f32, kind="ExternalInput")   # (dt*W).T tiles
    mt_in = nc.dram_tensor("mt", [2, P, C], f32, kind="ExternalInput")   # (dt*M).T tiles
    ht_in = nc.dram_tensor("ht", [2, P, C], f32, kind="ExternalInput")   # H.T tiles
    id_in = nc.dram_tensor("ident", [P, P], f32, kind="ExternalInput")
    b_in = nc.dram_tensor("bvec", [P, 2], f32, kind="ExternalInput")
    ys_out = nc.dram_tensor("ys", [INQ, C, T], f32, kind="ExternalOutput")
    xs_out = nc.dram_tensor("xs", [INQ, C, T], f32, kind="ExternalOutput")

    blocks_per_inq = T // B

    def dram3(t, blk):
        iq, tb = divmod(blk, blocks_per_inq)
        t0 = tb * B
        return t[iq].rearrange("(kh p) t -> p kh t", p=P)[:, :, t0 : t0 + B]

    with TileContext(nc) as tc:
        with (
            tc.tile_pool(name="const", bufs=1) as cpool,
            tc.tile_pool(name="io", bufs=3) as iopool,
            tc.tile_pool(name="xp", bufs=2) as xpool,
            tc.tile_pool(name="txp", bufs=2) as txpool,
            tc.tile_pool(name="pc", bufs=2, space="PSUM") as pcpool,
            tc.tile_pool(name="pv", bufs=2, space="PSUM") as pvpool,
            tc.tile_pool(name="py", bufs=2, space="PSUM") as pypool,
        ):
            w_sb = cpool.tile([P, 2, C], f32, tag="w")
            m_sb = cpool.tile([P, 2, C], f32, tag="m")
            h_sb = cpool.tile([P, 2, C], f32, tag="h")
            i_sb = cpool.tile([P, P], f32, tag="i")
            b_sb = cpool.tile([P, 2], f32, tag="b")
            a_sb = cpool.tile([P, B], f32, tag="a")
            nc.sync.dma_start(w_sb[:, :, :], wt_in.rearrange("a p f -> p a f"))
            nc.sync.dma_start(m_sb[:, :, :], mt_in.rearrange("a p f -> p a f"))
            nc.sync.dma_start(h_sb[:, :, :], ht_in.rearrange("a p f -> p a f"))
            nc.sync.dma_start(i_sb[:, :], id_in[:, :])
            nc.sync.dma_start(b_sb[:, :], b_in[:, :])
            nc.vector.memset(a_sb[:, :], a)

            # ---- per-block stage emitters ------------------------------
            def emit_load(blk):
                u_t = iopool.tile([P, 2, B], f32, tag="u")
                nc.sync.dma_start(u_t[:, :, :], dram3(u_in, blk))
                return u_t

            def emit_drive(u_t):
                # V = (dt*M) @ U into psum, plus an SBUF copy for reuse
                pv = pvpool.tile([P, 2 * B], f32, tag="pv")
                for mh in range(2):
                    for kh in range(2):
                        nc.tensor.matmul(
                            pv[:, mh * B : (mh + 1) * B],
                            m_sb[:, kh, mh * P : (mh + 1) * P],
                            u_t[:, kh, :],
                            start=(kh == 0),
                            stop=(kh == 1),
                        )
                v_t = iopool.tile([P, 2 * B], f32, tag="v")
                nc.scalar.copy(v_t[:, :], pv[:, :])
                return pv, v_t

            def scan(x_t, h, data1, initial):
                nc.vector.tensor_tensor_scan(
                    x_t[:, h * (B + 1) + 1 : h * (B + 1) + B + 1],
                    a_sb[:, :],
                    data1,
                    initial,
                    ALU.mult,
                    ALU.add,
                )

            def emit_scan0(blk, pv, x_prev):
                # iteration 0: x^0 = affine scan of the drive alone (G = 0).
                # scan initial reads the previous block's final state in place;
                # the x0 column used by tanh is copied on ACT in parallel.
                x_t = xpool.tile([P, 2 * (B + 1)], f32, tag="x")
                for h in range(2):
                    if blk == 0:
                        nc.vector.memset(x_t[:, h * (B + 1) : h * (B + 1) + 1], 0.0)
                        init = x_t[:, h * (B + 1) : h * (B + 1) + 1]
                    else:
                        init = x_prev[:, h * (B + 1) + B : h * (B + 1) + B + 1]
                        nc.scalar.copy(x_t[:, h * (B + 1) : h * (B + 1) + 1], init)
                    scan(x_t, h, pv[:, h * B : (h + 1) * B], init)
                return x_t

            def emit_iter(x_t, v_t):
                tx = txpool.tile([P, 2 * B], f32, tag="tx")
                pc = [
                    pcpool.tile([P, B], f32, tag=f"pc{mh}", name=f"pc{mh}")
                    for mh in range(2)
                ]
                # drive injection first: no dependence on this iteration
                for mh in range(2):
                    nc.tensor.matmul(
                        pc[mh][:, :],
                        i_sb[:, :],
                        v_t[:, mh * B : (mh + 1) * B],
                        start=True,
                        stop=False,
                    )
                # per contraction half: tanh of the shifted trajectory, then
                # its two matmuls — overlaps with the other half's tanh/scan
                for kh in range(2):
                    nc.scalar.activation(
                        tx[:, kh * B : (kh + 1) * B],
                        x_t[:, kh * (B + 1) : kh * (B + 1) + B],
                        AF.Tanh,
                    )
                    for mh in range(2):
                        nc.tensor.matmul(
                            pc[mh][:, :],
                            w_sb[:, kh, mh * P : (mh + 1) * P],
                            tx[:, kh * B : (kh + 1) * B],
                            start=False,
                            stop=(kh == 1),
                        )
                for h in range(2):
                    scan(
                        x_t,
                        h,
                        pc[h][:, :],
                        x_t[:, h * (B + 1) : h * (B + 1) + 1],
                    )

            def emit_readout(blk, x_t):
                py = pypool.tile([P, 2 * B], f32, tag="py")
                for mh in range(2):
                    for kh in range(2):
                        nc.tensor.matmul(
                            py[:, mh * B : (mh + 1) * B],
                            h_sb[:, kh, mh * P : (mh + 1) * P],
                            x_t[:, kh * (B + 1) + 1 : kh * (B + 1) + B + 1],
                            start=(kh == 0),
                            stop=(kh == 1),
                        )
                y_t = iopool.tile([P, 2, B], f32, tag="y")
                for mh in range(2):
                    nc.scalar.activation(
                        y_t[:, mh, :],
                        py[:, mh * B : (mh + 1) * B],
                        AF.Sigmoid,
                        bias=b_sb[:, mh : mh + 1],
                    )
                nc.sync.dma_start(dram3(ys_out, blk), y_t[:, :, :])
                x3 = x_t[:, :].rearrange("p (h t) -> p h t", h=2)
                nc.sync.dma_start(dram3(xs_out, blk), x3[:, :, 1 : B + 1])

            # ---- software-pipelined emission over blocks ---------------
            u_next = emit_load(0)
            pv_cur, v_cur = emit_drive(u_next)
            x_prev = None
            for blk in range(n_blocks):
                if blk + 1 < n_blocks:
                    u_next = emit_load(blk + 1)
                x_t = emit_scan0(blk, pv_cur, x_prev)
                v_t = v_cur
                for k in range(kit):
                    emit_iter(x_t, v_t)
                    if k == 0 and blk + 1 < n_blocks:
                        pv_cur, v_cur = emit_drive(u_next)
                    if k == 1 and x_prev is not None:
                        emit_readout(blk - 1, x_prev)
                x_prev = x_t
            emit_readout(n_blocks - 1, x_prev)

    nc.compile()
    return nc


def _get_nc(dt: float, n_blocks: int, kit: int):
    key = (dt, n_blocks, kit)
    if key not in _cache:
        _cache[key] = _build(dt, n_blocks, kit)
    return _cache[key]


LAST_RESULTS = None  # BassKernelResults of the most recent run (for profiling)


def kernel(u, dt, W, M, H, b, _n_blocks=None, _trace=False):
    from concourse.bass_utils import run_bass_kernel_spmd

    dt_f = float(np.asarray(dt).reshape(-1)[0])
    n_blocks = INQ * T // B if _n_blocks is None else _n_blocks
    nc = _get_nc(dt_f, n_blocks, KIT)

    W = np.asarray(W, np.float32)
    M = np.asarray(M, np.float32)
    H = np.asarray(H, np.float32)
    in_map = {
        "u": np.ascontiguousarray(np.asarray(u, np.float32)),
        "wt": np.ascontiguousarray((dt_f * W).T.reshape(2, P, C)),
        "mt": np.ascontiguousarray((dt_f * M).T.reshape(2, P, C)),
        "ht": np.ascontiguousarray(H.T.reshape(2, P, C)),
        "ident": np.eye(P, dtype=np.float32),
        "bvec": np.ascontiguousarray(np.asarray(b, np.float32).reshape(2, P).T),
    }
    res = run_bass_kernel_spmd(nc, [in_map], core_ids=[0], trace=_trace)
    global LAST_RESULTS
    LAST_RESULTS = res
    out = res.results[0]
    return out["ys"], out["xs"]


# revision 8
# speedup vs baseline: 1.1377x; 1.1377x over previous
"""Trainium2 Bass kernel for the leaky-tanh recurrent network (CombinedModel).

Math (reference):
    u_seq = u.transpose(0,2,1).reshape(Iq*T, C)      # state persists across inquiries
    x_t = (1-dt)*x_{t-1} + (dt*W) @ tanh(x_{t-1}) + (dt*M) @ u_t
    y_t = sigmoid(H @ x_t + b)
    outputs, membranes = y, x   (each reshaped back to [Iq, C, T])

Strategy: the recurrence is sequential in time, but it can be solved in
blocks of B steps with Picard (fixed-point) iteration:
    - the input drive V = (dt*M) @ U is a big batched matmul (TensorE)
    - given a guess trajectory X, G = (dt*W) @ tanh(X_shifted) is batched
    - the affine recurrence x_t = a*x_{t-1} + (G_t + V_t) is ONE hardware
      instruction per 128 channels via the DVE's tensor_tensor_scan
    - iterate K times; measured numerically vs the exact sequential scan:
      abs err 1.9e-4 at K=7 / 2.4e-5 at K=8 (B=128), below the ~3e-4 fp32
      accumulation drift any fp32 implementation shows over 65536 steps
Everything stays in [channel, time] layout on chip; DMA access patterns
handle the [Iq, C, T] <-> [C, time] mapping directly, so no transposes.

The per-iteration serial chain (scan -> tanh -> matmul -> scan) is
pipelined at half-channel granularity: tanh/matmuls for channels 0-127
run while the scan/tanh for channels 128-255 are still in flight, and the
drive-injection matmul (I @ V, which has no data dependence on the
iteration) opens each PSUM accumulation group early.  The V-phase of the
next block and the readout of the previous block are emitted in the
middle of the current block's iteration stretch so the in-order engines
execute them inside the chain's dependency stalls.
"""

import numpy as np

INQ, C, T = 64, 256, 1024
B = 128            # picard block length (divides T)
KIT = 7            # picard refinement iterations per block
P = 128            # partitions; C = 2*P channel halves

_cache: dict = {}


def _build(dt: float, n_blocks: int, kit: int):
    import concourse.bacc as bacc
    import concourse.mybir as mybir
    from concourse.tile import TileContext

    f32 = mybir.dt.float32
    AF = mybir.ActivationFunctionType
    ALU = mybir.AluOpType
    a = 1.0 - dt

    nc = bacc.Bacc(None, target_bir_lowering=False, debug=False)
    u_in = nc.dram_tensor("u", [INQ, C, T], f32, kind="ExternalInput")
    wt_in = nc.dram_tensor("wt", [2, P, C], f32, kind="ExternalInput")   # (dt*W).T tiles
    mt_in = nc.dram_tensor("mt", [2, P, C], f32, kind="ExternalInput")   # (dt*M).T tiles
    ht_in = nc.dram_tensor("ht", [2, P, C], f32, kind="ExternalInput")   # H.T tiles
    id_in = nc.dram_tensor("ident", [P, P], f32, kind="ExternalInput")
    b_in = nc.dram_tensor("bvec", [P, 2], f32, kind="ExternalInput")
    ys_out = nc.dram_tensor("ys", [INQ, C, T], f32, kind="ExternalOutput")
    xs_out = nc.dram_tensor("xs", [INQ, C, T], f32, kind="ExternalOutput")

    blocks_per_inq = T // B

    def dram3(t, blk):
        iq, tb = divmod(blk, blocks_per_inq)
        t0 = tb * B
        return t[iq].rearrange("(kh p) t -> p kh t", p=P)[:, :, t0 : t0 + B]

    with TileContext(nc) as tc:
        with (
            tc.tile_pool(name="const", bufs=1) as cpool,
            tc.tile_pool(name="io", bufs=3) as iopool,
            tc.tile_pool(name="xp", bufs=2) as xpool,
            tc.tile_pool(name="txp", bufs=2) as txpool,
            tc.tile_pool(name="pc", bufs=2, space="PSUM") as pcpool,
            tc.tile_pool(name="pv", bufs=2, space="PSUM") as pvpool,
            tc.tile_pool(name="py", bufs=2, space="PSUM") as pypool,
        ):
            w_sb = cpool.tile([P, 2, C], f32, tag="w")
            m_sb = cpool.tile([P, 2, C], f32, tag="m")
            h_sb = cpool.tile([P, 2, C], f32, tag="h")
            i_sb = cpool.tile([P, P], f32, tag="i")
            b_sb = cpool.tile([P, 2], f32, tag="b")
            a_sb = cpool.tile([P, B], f32, tag="a")
            nc.sync.dma_start(w_sb[:, :, :], wt_in.rearrange("a p f -> p a f"))
            nc.sync.dma_start(m_sb[:, :, :], mt_in.rearrange("a p f -> p a f"))
            nc.sync.dma_start(h_sb[:, :, :], ht_in.rearrange("a p f -> p a f"))
            nc.sync.dma_start(i_sb[:, :], id_in[:, :])
            nc.sync.dma_start(b_sb[:, :], b_in[:, :])
            nc.vector.memset(a_sb[:, :], a)

            # ---- per-block stage emitters ------------------------------
            def emit_load(blk):
                u_t = iopool.tile([P, 2, B], f32, tag="u")
                nc.sync.dma_start(u_t[:, :, :], dram3(u_in, blk))
                return u_t

            def emit_drive(u_t):
                # V = (dt*M) @ U into psum, plus an SBUF copy for reuse
                pv = pvpool.tile([P, 2 * B], f32, tag="pv")
                for mh in range(2):
                    for kh in range(2):
                        nc.tensor.matmul(
                            pv[:, mh * B : (mh + 1) * B],
                            m_sb[:, kh, mh * P : (mh + 1) * P],
                            u_t[:, kh, :],
                            start=(kh == 0),
                            stop=(kh == 1),
                        )
                v_t = iopool.tile([P, 2 * B], f32, tag="v")
                nc.scalar.copy(v_t[:, :], pv[:, :])
                return pv, v_t

            def scan(x_t, h, data1, initial):
                nc.vector.tensor_tensor_scan(
                    x_t[:, h * (B + 1) + 1 : h * (B + 1) + B + 1],
                    a_sb[:, :],
                    data1,
                    initial,
                    ALU.mult,
                    ALU.add,
                )

            def emit_scan0(blk, pv, x_prev):
                # iteration 0: x^0 = affine scan of the drive alone (G = 0).
                # scan initial reads the previous block's final state in place;
                # the x0 column used by tanh is copied on ACT in parallel.
                x_t = xpool.tile([P, 2 * (B + 1)], f32, tag="x")
                for h in range(2):
                    if blk == 0:
                        nc.vector.memset(x_t[:, h * (B + 1) : h * (B + 1) + 1], 0.0)
                        init = x_t[:, h * (B + 1) : h * (B + 1) + 1]
                    else:
                        init = x_prev[:, h * (B + 1) + B : h * (B + 1) + B + 1]
                        nc.scalar.copy(x_t[:, h * (B + 1) : h * (B + 1) + 1], init)
                    scan(x_t, h, pv[:, h * B : (h + 1) * B], init)
                return x_t

            def open_group(v_t):
                # drive injection I @ V opens the PSUM accumulation group for
                # a refinement iteration.  It has no dependence on the
                # iteration chain, so it is emitted an iteration early and
                # executes inside the chain's stalls.
                pc = [
                    pcpool.tile([P, B], f32, tag=f"pc{mh}", name=f"pc{mh}")
                    for mh in range(2)
                ]
                for mh in range(2):
                    nc.tensor.matmul(
                        pc[mh][:, :],
                        i_sb[:, :],
                        v_t[:, mh * B : (mh + 1) * B],
                        start=True,
                        stop=False,
                    )
                return pc

            def emit_iter_body(x_t, pc):
                tx = txpool.tile([P, 2 * B], f32, tag="tx")
                # per contraction half: tanh of the shifted trajectory, then
                # its two matmuls — overlaps with the other half's tanh/scan
                for kh in range(2):
                    nc.scalar.activation(
                        tx[:, kh * B : (kh + 1) * B],
                        x_t[:, kh * (B + 1) : kh * (B + 1) + B],
                        AF.Tanh,
                    )
                    for mh in range(2):
                        nc.tensor.matmul(
                            pc[mh][:, :],
                            w_sb[:, kh, mh * P : (mh + 1) * P],
                            tx[:, kh * B : (kh + 1) * B],
                            start=False,
                            stop=(kh == 1),
                        )

            def emit_scans(x_t, pc):
                for h in range(2):
                    scan(
                        x_t,
                        h,
                        pc[h][:, :],
                        x_t[:, h * (B + 1) : h * (B + 1) + 1],
                    )

            def emit_readout(blk, x_t):
                py = pypool.tile([P, 2 * B], f32, tag="py")
                for mh in range(2):
                    for kh in range(2):
                        nc.tensor.matmul(
                            py[:, mh * B : (mh + 1) * B],
                            h_sb[:, kh, mh * P : (mh + 1) * P],
                            x_t[:, kh * (B + 1) + 1 : kh * (B + 1) + B + 1],
                            start=(kh == 0),
                            stop=(kh == 1),
                        )
                y_t = iopool.tile([P, 2, B], f32, tag="y")
                for mh in range(2):
                    nc.scalar.activation(
                        y_t[:, mh, :],
                        py[:, mh * B : (mh + 1) * B],
                        AF.Sigmoid,
                        bias=b_sb[:, mh : mh + 1],
                    )
                nc.sync.dma_start(dram3(ys_out, blk), y_t[:, :, :])
                x3 = x_t[:, :].rearrange("p (h t) -> p h t", h=2)
                nc.sync.dma_start(dram3(xs_out, blk), x3[:, :, 1 : B + 1])

            # ---- software-pipelined emission over blocks ---------------
            u_next = emit_load(0)
            pv_cur, v_cur = emit_drive(u_next)
            x_prev = None
            pc_pending = None
            for blk in range(n_blocks):
                if blk + 1 < n_blocks:
                    u_next = emit_load(blk + 1)
                x_t = emit_scan0(blk, pv_cur, x_prev)
                v_t = v_cur
                for k in range(kit):
                    if pc_pending is None:
                        pc_pending = open_group(v_t)
                    pc_cur, pc_pending = pc_pending, None
                    emit_iter_body(x_t, pc_cur)
                    if k == 0 and blk + 1 < n_blocks:
                        pv_cur, v_cur = emit_drive(u_next)
                    # open the next iteration's group before this one's scans
                    if k + 1 < kit:
                        pc_pending = open_group(v_t)
                    elif blk + 1 < n_blocks:
                        pc_pending = open_group(v_cur)
                    emit_scans(x_t, pc_cur)
                    if k == 1 and x_prev is not None:
                        emit_readout(blk - 1, x_prev)
                x_prev = x_t
            emit_readout(n_blocks - 1, x_prev)

    nc.compile()
    return nc


def _get_nc(dt: float, n_blocks: int, kit: int):
    key = (dt, n_blocks, kit)
    if key not in _cache:
        _cache[key] = _build(dt, n_blocks, kit)
    return _cache[key]


LAST_RESULTS = None  # BassKernelResults of the most recent run (for profiling)


def kernel(u, dt, W, M, H, b, _n_blocks=None, _trace=False):
    from concourse.bass_utils import run_bass_kernel_spmd

    dt_f = float(np.asarray(dt).reshape(-1)[0])
    n_blocks = INQ * T // B if _n_blocks is None else _n_blocks
    nc = _get_nc(dt_f, n_blocks, KIT)

    W = np.asarray(W, np.float32)
    M = np.asarray(M, np.float32)
    H = np.asarray(H, np.float32)
    in_map = {
        "u": np.ascontiguousarray(np.asarray(u, np.float32)),
        "wt": np.ascontiguousarray((dt_f * W).T.reshape(2, P, C)),
        "mt": np.ascontiguousarray((dt_f * M).T.reshape(2, P, C)),
        "ht": np.ascontiguousarray(H.T.reshape(2, P, C)),
        "ident": np.eye(P, dtype=np.float32),
        "bvec": np.ascontiguousarray(np.asarray(b, np.float32).reshape(2, P).T),
    }
    res = run_bass_kernel_spmd(nc, [in_map], core_ids=[0], trace=_trace)
    global LAST_RESULTS
    LAST_RESULTS = res
    out = res.results[0]
    return out["ys"], out["xs"]
